# revision 23
# baseline (speedup 1.0000x reference)
"""Gaussian-kernel attention for Trainium2 (Bass/Tile), 8-core data-parallel.

Computes out = x + K @ x with K = exp(-r * d2), d2[t,s] = ||x_t - x_s||^2,
per batch.  Decomposition used on-chip:

    d2 = sq_t + sq_s - 2*G          (G = X X^T, sq = rowwise |x|^2)
    K  = e_t * S * e_s              (S = exp(2r*G), e_i = exp(-r*sq_i))
    out[u] = x[u] + e_u * Z[u],  Z[u] = sum_v S[u,v] * (e_v x[v])

S is symmetric, which lets mm2 contract over the PARTITION dim of the
S-stripes directly (Z^T = sum_i Y_i^T S_i with S_i = stripe [v-block i,
all u]) -- no per-tile transposes anywhere in the main loop.

The work is ONE continuous stream of 512-wide chunks across all local
batches (chunk = [128 t, 512 s] of G/S), grouped into PSUM tiles of 3
chunks so each exp ACTIVATE covers N=1536 (ACT is the roofline engine;
bigger N amortizes its ~260ns per-instruction overhead, and the
software-pipeline skew across batch boundaries keeps ACT gapless).
Per batch the chunks run in two passes over the s/u quarters --
(q=0,1 for all stripes) then (q=2,3) -- so each output half's epilogue
runs as soon as its accumulation chains stop, hiding the tail.

PSUM budget (8 banks):
  g tiles [128, 3, 512] x2 bufs -- 6 banks, ping-pong
  pz      [128, 2, 512]         -- 2 banks, Z^T col-packed: 4 interleaved
          accumulation chains keyed by the s/u quarter (has_written clear
          is per-partition, verified on HW, so chains may share banks)
PE packing: mm1 (K=64) alternates row groups 0/64 per stripe.
The prologue (x load -> bf16 dup -> DRAM round-trip -> DMA-xbar X^T)
is pipelined at t-quarter granularity to cut batch-0 latency.

Sharding: pure data-parallel over batch B=32 -> 4 batches per core x 8.
"""

import os
import sys

import numpy as np

sys.path.insert(0, "/opt/trn_rl_repo")

import concourse.bass as bass
import concourse.tile as tile
from concourse import bacc, mybir
from concourse.bass_utils import run_bass_kernel_spmd

FP32 = mybir.dt.float32
BF16 = mybir.dt.bfloat16
FP16 = mybir.dt.float16

B, T, C = 32, 2048, 64
N_CORES = 8
BPC = B // N_CORES  # batches per core

# Stashed by kernel() for the test harness (exec time etc.)
LAST_RESULTS = None


def _body(ctx, tc, out_ap, x_ap, r, bpc, t, dbg=False):
    """Emit the per-core kernel IR.

    out_ap/x_ap: DRAM APs of shape [bpc, t, C].
    r: python float (r_sigma value, baked as immediates).
    """
    nc = tc.nc

    def dump(name, sb_ap, dt=None):
        if not dbg:
            return
        d = nc.dram_tensor(
            name, list(sb_ap.shape), dt or sb_ap.dtype, kind="ExternalOutput"
        ).ap()
        nc.sync.dma_start(out=d, in_=sb_ap)

    nt = t // 128          # t-blocks (stripes) per batch
    nq = nt // 4           # t-blocks per quarter
    exp2r = 2.0 * r

    # degree-7 polynomial fit of e^z on [-2, 2] (|z|=|2r G| stays < ~1 for
    # this problem's r=0.01, randn x); used by the DVE-offloaded exp tiles.
    # Horner via scalar_tensor_tensor: p=a7*z; p=(p+a_k)*z ...; out=p+a0.
    zg = np.linspace(-2.0, 2.0, 4001)
    pc_hi_first = np.polyfit(zg, np.exp(zg), 7)
    PA = [float(v) for v in pc_hi_first]  # [a7, a6, ..., a0]

    # chunk stream: per batch, pass 0 = quarters (0, 1), pass 1 = (2, 3)
    chunks = [
        (b, i, q)
        for b in range(bpc)
        for qp in range(2)
        for i in range(nt)
        for q in (2 * qp, 2 * qp + 1)
    ]
    nck = len(chunks)
    ntile = (nck + 2) // 3

    # tile index after whose mm2 each (batch, half) epilogue can run:
    # the tile containing that half's last chunk (i = nt-1)
    epi_after = {}
    for pos, (b, i, q) in enumerate(chunks):
        if i == nt - 1 and q % 2 == 1:
            epi_after[pos // 3] = (b, q // 2)
    # stream position at which to emit each batch's prologue (lookahead)
    prolog_at = {}
    for b in range(bpc):
        first = min(pos for pos, c in enumerate(chunks) if c[0] == b) // 3
        prolog_at[b] = max(0, first - 8)

    # exp tiles offloaded from ACT (roofline engine) to a DVE polynomial;
    # spaced out and kept away from epilogue tiles so the DVE FIFO never
    # delays an epilogue's PSUM drain.
    avoid = set()
    for e in epi_after:
        avoid.update((e - 1, e, e + 1, e + 2))
    dve_tiles = set()
    kx = 5
    while kx < ntile - 2 and len(dve_tiles) < 9:
        for cand in (kx, kx + 1, kx - 1, kx + 2, kx + 3):
            if cand not in avoid and 3 <= cand < ntile - 1 and cand not in dve_tiles:
                dve_tiles.add(cand)
                break
        kx += 9

    # SBUF pools (2 slots per tag for cross-batch pipelining)
    xpool = ctx.enter_context(tc.tile_pool(name="x32", bufs=2))
    xxpool = ctx.enter_context(tc.tile_pool(name="xx", bufs=2))
    sqpool = ctx.enter_context(tc.tile_pool(name="sq", bufs=2))
    ypool = ctx.enter_context(tc.tile_pool(name="yb", bufs=2))
    xbpool = ctx.enter_context(tc.tile_pool(name="xbp", bufs=2))
    xtpool = ctx.enter_context(tc.tile_pool(name="xt", bufs=2))
    apool = ctx.enter_context(tc.tile_pool(name="a0", bufs=6))
    opool = ctx.enter_context(tc.tile_pool(name="osb", bufs=2))
    zpool = ctx.enter_context(tc.tile_pool(name="zply", bufs=2))
    # PSUM: 2 ping-pong G tiles of 3 banks + col-packed Z^T accum (2 banks)
    gpool = ctx.enter_context(tc.tile_pool(name="gps", bufs=2, space="PSUM"))
    ppool = ctx.enter_context(tc.tile_pool(name="pps", bufs=1, space="PSUM"))
    # DRAM scratch for the bf16 transpose round-trip
    dpool = ctx.enter_context(tc.tile_pool(name="dsc", bufs=2, space="DRAM"))

    pz = ppool.tile([128, 2, 512], FP32)    # 2 banks, Z^T col-packed

    # per-batch state, filled by prologue(b)
    st = [None] * bpc

    def prologue(b):
        xb_dram = x_ap[b].rearrange("(k p) c -> p k c", p=128)  # [128, nt, C]
        x32, xth = [], []
        for u in range(4):
            xh = xpool.tile([128, nq, C], FP32, tag=f"x32{u}")
            nc.sync.dma_start(out=xh[:], in_=xb_dram[:, u * nq : (u + 1) * nq])
            x32.append(xh)
            # bf16 x, written twice side by side, so one DMA-xbar transpose
            # yields X^T duplicated on both partition halves (mm1 row groups)
            xbp = xbpool.tile([128, nq, 2 * C], BF16, tag=f"xbp{u}")
            nc.vector.tensor_copy(xbp[:, :, 0:C], xh[:])
            nc.vector.tensor_copy(xbp[:, :, C : 2 * C], xh[:])
            xbd = dpool.tile([t // 4, 2 * C], BF16, tag=f"xbd{u}")
            nc.sync.dma_start(
                out=xbd.rearrange("(k p) c -> p k c", p=128), in_=xbp[:]
            )
            xt = xtpool.tile([128, t // 4], BF16, tag=f"xt{u}")
            nc.sync.dma_start_transpose(out=xt[:], in_=xbd[:])
            xth.append(xt)
            # xt[c, tt] = xt[64+c, tt] = x[u*512 + tt, c] for c < 64.

        # row stats per quarter -- sq/ev/yb chains stay quarter-local so the
        # first mm2 never waits on the whole batch's x to load.  They run on
        # the otherwise-idle GpSimd engine to keep the DVE queue short.
        ev, yb = [], []
        for u in range(4):
            xx = xxpool.tile([128, nq, C], FP32, tag=f"xx{u % 2}")
            nc.vector.tensor_mul(xx[:], x32[u][:], x32[u][:])
            sq = sqpool.tile([128, nq], FP32, tag=f"sq{u}")
            nc.vector.tensor_reduce(
                sq[:],
                xx[:],
                axis=mybir.AxisListType.X,
                op=mybir.AluOpType.add,
            )
            evu = sqpool.tile([128, nq], FP32, tag=f"ev{u}")
            nc.scalar.activation(
                evu[:], sq[:], mybir.ActivationFunctionType.Exp, scale=-r
            )
            ev.append(evu)
            ybu = ypool.tile([128, nq, C], BF16, tag=f"yb{u}")
            for k in range(nq):
                nc.vector.tensor_scalar_mul(
                    ybu[:, k], x32[u][:, k], evu[:, k : k + 1]
                )
            yb.append(ybu)
        st[b] = (x32, xth, ev, yb)
        if dbg and b == 0:
            dump("dbg_ev", ev[0][:])
            dump("dbg_yb", yb[0][:])
            dump("dbg_xt", xth[0][:])

    a0_tiles = [None] * ntile
    g_tiles = [None] * ntile

    def nch(k):
        return min(3, nck - 3 * k)

    def mm1(k):
        g = gpool.tile([128, 3, 512], FP32)
        g_tiles[k] = g
        for j in range(nch(k)):
            b, i, q = chunks[3 * k + j]
            xth = st[b][1]
            rows = 64 * (i % 2)
            xl = xth[i // nq]         # lhsT t-block quarter
            nc.tensor.matmul(
                g[:, j],
                lhsT=xl[rows : rows + 64, (i % nq) * 128 : (i % nq + 1) * 128],
                rhs=xth[q][rows : rows + 64, :],
                start=True,
                stop=True,
                skip_group_check=True,
            )

    def act(k):
        n = nch(k)
        g = g_tiles[k]
        g_tiles[k] = None
        a0 = apool.tile([128, 3, 512], BF16)
        if k in dve_tiles:
            # DVE polynomial exp: z = 2r*g (frees the PSUM tile fast),
            # then Horner in fp16 at the DVE 2x rate.
            z = zpool.tile([128, 3, 512], FP16, tag="z")
            nc.vector.tensor_scalar_mul(z[:, 0:n], g[:, 0:n], exp2r)
            pa = zpool.tile([128, 3, 512], FP16, tag="pa")
            pb = zpool.tile([128, 3, 512], FP16, tag="pb")
            nc.vector.tensor_scalar_mul(pa[:, 0:n], z[:, 0:n], PA[0])
            cur, nxt = pa, pb
            for a_k in PA[1:7]:
                nc.vector.scalar_tensor_tensor(
                    nxt[:, 0:n],
                    in0=cur[:, 0:n],
                    scalar=a_k,
                    in1=z[:, 0:n],
                    op0=mybir.AluOpType.add,
                    op1=mybir.AluOpType.mult,
                )
                cur, nxt = nxt, cur
            nc.vector.tensor_scalar_add(a0[:, 0:n], cur[:, 0:n], PA[7])
        else:
            nc.scalar.activation(
                a0[:, 0:n],
                g[:, 0:n],
                mybir.ActivationFunctionType.Exp,
                scale=exp2r,
            )
        a0_tiles[k] = a0
        if dbg and k == 0:
            gsb = xxpool.tile([128, 3, 512], FP32, tag="gdump")
            nc.vector.tensor_copy(gsb[:], g[:])
            dump("dbg_g00", gsb[:])
            dump("dbg_a00", a0[:])

    def mm2(k):
        a0 = a0_tiles[k]
        a0_tiles[k] = None
        for j in range(nch(k)):
            b, i, q = chunks[3 * k + j]
            yb = st[b][3]
            # chain q: parts 64*(q//2).., bank q%2; start/stop per stripe
            nc.tensor.matmul(
                pz[64 * (q // 2) : 64 * (q // 2) + 64, q % 2],
                lhsT=yb[i // nq][:, i % nq],
                rhs=a0[:, j],
                start=(i == 0),
                stop=(i == nt - 1),
                skip_group_check=True,
            )

    # ---- epilogue (one output half, two u-quarters pipelined):
    # Z^T -> bf16 -> DMA-xbar transpose -> e_u * Z + x -> DRAM.
    # tr[p, jj, c] = zq[64h+c, jj*128+p] = Z[u, c] for u-block 4*q+jj
    def epilogue(b, h):
        x32, _, ev, _ = st[b]
        ob_dram = out_ap[b].rearrange("(k p) c -> p k c", p=128)
        for n in range(2):
            q = 2 * h + n
            zq = opool.tile([128, 512], BF16, tag=f"zt{q}")
            nc.vector.tensor_copy(
                zq[64 * h : 64 * h + 64, :], pz[64 * h : 64 * h + 64, n]
            )
            tr = opool.tile([128, nq, C], BF16, tag=f"tr{q}")
            nc.sync.dma_start_transpose(
                out=tr[:], in_=zq[64 * h : 64 * h + 64, :]
            )
            osb = opool.tile([128, nq, C], FP32, tag=f"osb{q}")
            for jj in range(nq):
                j = q * nq + jj
                nc.vector.scalar_tensor_tensor(
                    osb[:, jj],
                    in0=tr[:, jj],
                    scalar=ev[j // nq][:, j % nq : j % nq + 1],
                    in1=x32[j // nq][:, j % nq],
                    op0=mybir.AluOpType.mult,
                    op1=mybir.AluOpType.add,
                )
            nc.sync.dma_start(
                out=ob_dram[:, q * nq : (q + 1) * nq], in_=osb[:]
            )

    # ---- main stream, software-pipelined emission: step k emits
    # mm2(k-2) -> mm1(k) -> ACT(k-1) so the PE queue head never blocks
    # on fresh work, including across batch boundaries. ----
    for k in range(ntile + 3):
        for b, pos in prolog_at.items():
            if pos == k:
                prologue(b)
        if k >= 3:
            mm2(k - 3)
            if k - 3 in epi_after:
                epilogue(*epi_after[k - 3])
        if k < ntile:
            mm1(k)
        if 1 <= k < ntile + 1:
            act(k - 1)


def build(r, bpc=BPC, t=T, dbg=False):
    """Build + compile the Bass module for one core's shard."""
    from contextlib import ExitStack

    nc = bacc.Bacc(
        "TRN2", target_bir_lowering=False, debug=False, num_devices=N_CORES
    )
    x_ap = nc.dram_tensor("x", [bpc, t, C], FP32, kind="ExternalInput").ap()
    out_ap = nc.dram_tensor("out", [bpc, t, C], FP32, kind="ExternalOutput").ap()
    with tile.TileContext(nc) as tc:
        with ExitStack() as ctx:
            _body(ctx, tc, out_ap, x_ap, r, bpc, t, dbg=dbg)
    nc.compile()
    return nc


def kernel(x, r_sigma):
    global LAST_RESULTS
    x = np.ascontiguousarray(np.asarray(x, dtype=np.float32))
    r = float(np.asarray(r_sigma).reshape(-1)[0])
    assert x.shape == (B, T, C), x.shape

    nc = build(r)
    in_maps = [
        {"x": np.ascontiguousarray(x[i * BPC : (i + 1) * BPC])}
        for i in range(N_CORES)
    ]
    trace = bool(int(os.environ.get("KERNEL_TRACE", "0")))
    res = run_bass_kernel_spmd(
        nc, in_maps, core_ids=list(range(N_CORES)), trace=trace
    )
    LAST_RESULTS = res
    out = np.concatenate([res.results[i]["out"] for i in range(N_CORES)], axis=0)
    return out.astype(np.float32)


# revision 25
# speedup vs baseline: 1.7252x; 1.7252x over previous
"""Gaussian-kernel attention for Trainium2 (Bass/Tile), 8-core data-parallel.

Computes out = x + K @ x with K = exp(-r * d2), d2[t,s] = ||x_t - x_s||^2,
per batch.  Decomposition used on-chip:

    d2 = sq_t + sq_s - 2*G          (G = X X^T, sq = rowwise |x|^2)
    K  = e_t * S * e_s              (S = exp(2r*G), e_i = exp(-r*sq_i))
    out[u] = x[u] + e_u * Z[u],  Z[u] = sum_v S[u,v] * (e_v x[v])

S is symmetric, which lets mm2 contract over the PARTITION dim of the
S-stripes directly (Z^T = sum_i Y_i^T S_i with S_i = stripe [v-block i,
all u]) -- no per-tile transposes anywhere in the main loop.

The work is ONE continuous stream of 512-wide chunks across all local
batches (chunk = [128 t, 512 s] of G/S), grouped into PSUM tiles of 3
chunks so each exp ACTIVATE covers N=1536 (ACT is the roofline engine;
bigger N amortizes its ~260ns per-instruction overhead, and the
software-pipeline skew across batch boundaries keeps ACT gapless).
Per batch the chunks run in two passes over the s/u quarters --
(q=0,1 for all stripes) then (q=2,3) -- so each output half's epilogue
runs as soon as its accumulation chains stop, hiding the tail.

PSUM budget (8 banks):
  g tiles [128, 3, 512] x2 bufs -- 6 banks, ping-pong
  pz      [128, 2, 512]         -- 2 banks, Z^T col-packed: 4 interleaved
          accumulation chains keyed by the s/u quarter (has_written clear
          is per-partition, verified on HW, so chains may share banks)
PE packing: mm1 (K=64) alternates row groups 0/64 per stripe.
The prologue (x load -> bf16 dup -> DRAM round-trip -> DMA-xbar X^T)
is pipelined at t-quarter granularity to cut batch-0 latency.

Sharding: pure data-parallel over batch B=32 -> 4 batches per core x 8.
"""

import os
import sys

import numpy as np

sys.path.insert(0, "/opt/trn_rl_repo")

import concourse.bass as bass
import concourse.tile as tile
from concourse import bacc, mybir
from concourse.bass_utils import run_bass_kernel_spmd

FP32 = mybir.dt.float32
BF16 = mybir.dt.bfloat16
FP16 = mybir.dt.float16

B, T, C = 32, 2048, 64
N_CORES = 8
BPC = B // N_CORES  # batches per core

# Stashed by kernel() for the test harness (exec time etc.)
LAST_RESULTS = None


def _body(ctx, tc, out_ap, x_ap, r, bpc, t, dbg=False):
    """Emit the per-core kernel IR.

    out_ap/x_ap: DRAM APs of shape [bpc, t, C].
    r: python float (r_sigma value, baked as immediates).
    """
    nc = tc.nc

    def dump(name, sb_ap, dt=None):
        if not dbg:
            return
        d = nc.dram_tensor(
            name, list(sb_ap.shape), dt or sb_ap.dtype, kind="ExternalOutput"
        ).ap()
        nc.sync.dma_start(out=d, in_=sb_ap)

    nt = t // 128          # t-blocks (stripes) per batch
    nq = nt // 4           # t-blocks per quarter
    exp2r = 2.0 * r

    # degree-7 polynomial fit of e^z on [-2, 2] (|z|=|2r G| stays < ~1 for
    # this problem's r=0.01, randn x); used by the DVE-offloaded exp tiles.
    # Horner via scalar_tensor_tensor: p=a7*z; p=(p+a_k)*z ...; out=p+a0.
    zg = np.linspace(-2.0, 2.0, 4001)
    pc_hi_first = np.polyfit(zg, np.exp(zg), 7)
    PA = [float(v) for v in pc_hi_first]  # [a7, a6, ..., a0]

    # chunk stream: per batch, pass 0 = quarters (0, 1), pass 1 = (2, 3)
    chunks = [
        (b, i, q)
        for b in range(bpc)
        for qp in range(2)
        for i in range(nt)
        for q in (2 * qp, 2 * qp + 1)
    ]
    nck = len(chunks)
    ntile = (nck + 2) // 3

    # tile index after whose mm2 each (batch, half) epilogue can run:
    # the tile containing that half's last chunk (i = nt-1)
    epi_after = {}
    for pos, (b, i, q) in enumerate(chunks):
        if i == nt - 1 and q % 2 == 1:
            epi_after[pos // 3] = (b, q // 2)
    # stream position at which to emit each batch's prologue (lookahead)
    prolog_at = {}
    for b in range(bpc):
        first = min(pos for pos, c in enumerate(chunks) if c[0] == b) // 3
        prolog_at[b] = max(0, first - 12)

    # exp tiles offloaded from ACT (roofline engine) to a DVE polynomial;
    # spaced out and kept away from epilogue tiles so the DVE FIFO never
    # delays an epilogue's PSUM drain.
    avoid = set()
    for e in epi_after:
        avoid.update((e - 1, e, e + 1, e + 2))
    # (measured: a full-tile DVE polynomial blob (~9us) exceeds the pipeline
    # skew and stalls mm2 -- net large loss, so the offload stays disabled)
    dve_tiles = set()

    # SBUF pools (2 slots per tag for cross-batch pipelining)
    xpool = ctx.enter_context(tc.tile_pool(name="x32", bufs=2))
    xxpool = ctx.enter_context(tc.tile_pool(name="xx", bufs=2))
    sqpool = ctx.enter_context(tc.tile_pool(name="sq", bufs=2))
    ypool = ctx.enter_context(tc.tile_pool(name="yb", bufs=2))
    xbpool = ctx.enter_context(tc.tile_pool(name="xbp", bufs=2))
    xtpool = ctx.enter_context(tc.tile_pool(name="xt", bufs=2))
    apool = ctx.enter_context(tc.tile_pool(name="a0", bufs=6))
    opool = ctx.enter_context(tc.tile_pool(name="osb", bufs=2))
    zpool = ctx.enter_context(tc.tile_pool(name="zply", bufs=2))
    # PSUM: 2 ping-pong G tiles of 3 banks + col-packed Z^T accum (2 banks)
    gpool = ctx.enter_context(tc.tile_pool(name="gps", bufs=2, space="PSUM"))
    ppool = ctx.enter_context(tc.tile_pool(name="pps", bufs=1, space="PSUM"))
    # DRAM scratch for the bf16 transpose round-trip
    dpool = ctx.enter_context(tc.tile_pool(name="dsc", bufs=2, space="DRAM"))

    pz = ppool.tile([128, 2, 512], FP32)    # 2 banks, Z^T col-packed

    # per-batch state, filled by prologue(b)
    st = [None] * bpc

    def prologue(b):
        xb_dram = x_ap[b].rearrange("(k p) c -> p k c", p=128)  # [128, nt, C]
        x32, xth = [], []
        for u in range(4):
            xh = xpool.tile([128, nq, C], FP32, tag=f"x32{u}")
            nc.sync.dma_start(out=xh[:], in_=xb_dram[:, u * nq : (u + 1) * nq])
            x32.append(xh)
            # bf16 x, written twice side by side, so one DMA-xbar transpose
            # yields X^T duplicated on both partition halves (mm1 row groups)
            xbp = xbpool.tile([128, nq, 2 * C], BF16, tag=f"xbp{u}")
            nc.vector.tensor_copy(xbp[:, :, 0:C], xh[:])
            nc.vector.tensor_copy(xbp[:, :, C : 2 * C], xh[:])
            xbd = dpool.tile([t // 4, 2 * C], BF16, tag=f"xbd{u}")
            nc.sync.dma_start(
                out=xbd.rearrange("(k p) c -> p k c", p=128), in_=xbp[:]
            )
            xt = xtpool.tile([128, t // 4], BF16, tag=f"xt{u}")
            nc.sync.dma_start_transpose(out=xt[:], in_=xbd[:])
            xth.append(xt)
            # xt[c, tt] = xt[64+c, tt] = x[u*512 + tt, c] for c < 64.

        # row stats per quarter -- sq/ev/yb chains stay quarter-local so the
        # first mm2 never waits on the whole batch's x to load.  They run on
        # the otherwise-idle GpSimd engine to keep the DVE queue short.
        ev, yb = [], []
        for u in range(4):
            xx = xxpool.tile([128, nq, C], FP32, tag=f"xx{u % 2}")
            nc.vector.tensor_mul(xx[:], x32[u][:], x32[u][:])
            sq = sqpool.tile([128, nq], FP32, tag=f"sq{u}")
            nc.vector.tensor_reduce(
                sq[:],
                xx[:],
                axis=mybir.AxisListType.X,
                op=mybir.AluOpType.add,
            )
            evu = sqpool.tile([128, nq], FP32, tag=f"ev{u}")
            nc.scalar.activation(
                evu[:], sq[:], mybir.ActivationFunctionType.Exp, scale=-r
            )
            ev.append(evu)
            ybu = ypool.tile([128, nq, C], BF16, tag=f"yb{u}")
            for k in range(nq):
                nc.vector.tensor_scalar_mul(
                    ybu[:, k], x32[u][:, k], evu[:, k : k + 1]
                )
            yb.append(ybu)
        st[b] = (x32, xth, ev, yb)
        if dbg and b == 0:
            dump("dbg_ev", ev[0][:])
            dump("dbg_yb", yb[0][:])
            dump("dbg_xt", xth[0][:])

    a0_tiles = [None] * ntile
    g_tiles = [None] * ntile

    def nch(k):
        return min(3, nck - 3 * k)

    def mm1(k):
        g = gpool.tile([128, 3, 512], FP32)
        g_tiles[k] = g
        for j in range(nch(k)):
            b, i, q = chunks[3 * k + j]
            xth = st[b][1]
            rows = 64 * (i % 2)
            xl = xth[i // nq]         # lhsT t-block quarter
            nc.tensor.matmul(
                g[:, j],
                lhsT=xl[rows : rows + 64, (i % nq) * 128 : (i % nq + 1) * 128],
                rhs=xth[q][rows : rows + 64, :],
                start=True,
                stop=True,
                skip_group_check=True,
            )

    def act(k):
        n = nch(k)
        g = g_tiles[k]
        g_tiles[k] = None
        a0 = apool.tile([128, 3, 512], BF16)
        if k in dve_tiles:
            # DVE polynomial exp: z = 2r*g (frees the PSUM tile fast),
            # then Horner in fp16 at the DVE 2x rate.
            z = zpool.tile([128, 3, 512], FP16, tag="z")
            nc.vector.tensor_scalar_mul(z[:, 0:n], g[:, 0:n], exp2r)
            pa = zpool.tile([128, 3, 512], FP16, tag="pa")
            pb = zpool.tile([128, 3, 512], FP16, tag="pb")
            nc.vector.tensor_scalar_mul(pa[:, 0:n], z[:, 0:n], PA[0])
            cur, nxt = pa, pb
            for a_k in PA[1:7]:
                nc.vector.scalar_tensor_tensor(
                    nxt[:, 0:n],
                    in0=cur[:, 0:n],
                    scalar=a_k,
                    in1=z[:, 0:n],
                    op0=mybir.AluOpType.add,
                    op1=mybir.AluOpType.mult,
                )
                cur, nxt = nxt, cur
            nc.vector.tensor_scalar_add(a0[:, 0:n], cur[:, 0:n], PA[7])
        else:
            nc.scalar.activation(
                a0[:, 0:n],
                g[:, 0:n],
                mybir.ActivationFunctionType.Exp,
                scale=exp2r,
            )
        a0_tiles[k] = a0
        if dbg and k == 0:
            gsb = xxpool.tile([128, 3, 512], FP32, tag="gdump")
            nc.vector.tensor_copy(gsb[:], g[:])
            dump("dbg_g00", gsb[:])
            dump("dbg_a00", a0[:])

    def mm2(k):
        a0 = a0_tiles[k]
        a0_tiles[k] = None
        for j in range(nch(k)):
            b, i, q = chunks[3 * k + j]
            yb = st[b][3]
            # chain q: parts 64*(q//2).., bank q%2; start/stop per stripe
            nc.tensor.matmul(
                pz[64 * (q // 2) : 64 * (q // 2) + 64, q % 2],
                lhsT=yb[i // nq][:, i % nq],
                rhs=a0[:, j],
                start=(i == 0),
                stop=(i == nt - 1),
                skip_group_check=True,
            )

    # ---- epilogue (one output half, two u-quarters pipelined):
    # Z^T -> bf16 -> DMA-xbar transpose -> e_u * Z + x -> DRAM.
    # tr[p, jj, c] = zq[64h+c, jj*128+p] = Z[u, c] for u-block 4*q+jj
    def epilogue(b, h):
        x32, _, ev, _ = st[b]
        ob_dram = out_ap[b].rearrange("(k p) c -> p k c", p=128)
        for n in range(2):
            q = 2 * h + n
            zq = opool.tile([128, 512], BF16, tag=f"zt{q}")
            nc.vector.tensor_copy(
                zq[64 * h : 64 * h + 64, :], pz[64 * h : 64 * h + 64, n]
            )
            tr = opool.tile([128, nq, C], BF16, tag=f"tr{q}")
            nc.sync.dma_start_transpose(
                out=tr[:], in_=zq[64 * h : 64 * h + 64, :]
            )
            osb = opool.tile([128, nq, C], FP32, tag=f"osb{q}")
            for jj in range(nq):
                j = q * nq + jj
                nc.vector.scalar_tensor_tensor(
                    osb[:, jj],
                    in0=tr[:, jj],
                    scalar=ev[j // nq][:, j % nq : j % nq + 1],
                    in1=x32[j // nq][:, j % nq],
                    op0=mybir.AluOpType.mult,
                    op1=mybir.AluOpType.add,
                )
            nc.sync.dma_start(
                out=ob_dram[:, q * nq : (q + 1) * nq], in_=osb[:]
            )

    # ---- main stream, software-pipelined emission: step k emits
    # mm2(k-2) -> mm1(k) -> ACT(k-1) so the PE queue head never blocks
    # on fresh work, including across batch boundaries. ----
    for k in range(ntile + 3):
        for b, pos in prolog_at.items():
            if pos == k:
                prologue(b)
        if k >= 3:
            mm2(k - 3)
            if k - 3 in epi_after:
                epilogue(*epi_after[k - 3])
        if k < ntile:
            mm1(k)
        if 1 <= k < ntile + 1:
            act(k - 1)


def build(r, bpc=BPC, t=T, dbg=False):
    """Build + compile the Bass module for one core's shard."""
    from contextlib import ExitStack

    nc = bacc.Bacc(
        "TRN2", target_bir_lowering=False, debug=False, num_devices=N_CORES
    )
    x_ap = nc.dram_tensor("x", [bpc, t, C], FP32, kind="ExternalInput").ap()
    out_ap = nc.dram_tensor("out", [bpc, t, C], FP32, kind="ExternalOutput").ap()
    with tile.TileContext(nc) as tc:
        with ExitStack() as ctx:
            _body(ctx, tc, out_ap, x_ap, r, bpc, t, dbg=dbg)
    nc.compile()
    return nc


def kernel(x, r_sigma):
    global LAST_RESULTS
    x = np.ascontiguousarray(np.asarray(x, dtype=np.float32))
    r = float(np.asarray(r_sigma).reshape(-1)[0])
    assert x.shape == (B, T, C), x.shape

    nc = build(r)
    in_maps = [
        {"x": np.ascontiguousarray(x[i * BPC : (i + 1) * BPC])}
        for i in range(N_CORES)
    ]
    trace = bool(int(os.environ.get("KERNEL_TRACE", "0")))
    res = run_bass_kernel_spmd(
        nc, in_maps, core_ids=list(range(N_CORES)), trace=trace
    )
    LAST_RESULTS = res
    out = np.concatenate([res.results[i]["out"] for i in range(N_CORES)], axis=0)
    return out.astype(np.float32)


# revision 26
# speedup vs baseline: 1.7493x; 1.0139x over previous
"""Gaussian-kernel attention for Trainium2 (Bass/Tile), 8-core data-parallel.

Computes out = x + K @ x with K = exp(-r * d2), d2[t,s] = ||x_t - x_s||^2,
per batch.  Decomposition used on-chip:

    d2 = sq_t + sq_s - 2*G          (G = X X^T, sq = rowwise |x|^2)
    K  = e_t * S * e_s              (S = exp(2r*G), e_i = exp(-r*sq_i))
    out[u] = x[u] + e_u * Z[u],  Z[u] = sum_v S[u,v] * (e_v x[v])

S is symmetric, which lets mm2 contract over the PARTITION dim of the
S-stripes directly (Z^T = sum_i Y_i^T S_i with S_i = stripe [v-block i,
all u]) -- no per-tile transposes anywhere in the main loop.

The work is ONE continuous stream of 512-wide chunks across all local
batches (chunk = [128 t, 512 s] of G/S), grouped into PSUM tiles of 3
chunks so each exp ACTIVATE covers N=1536 (ACT is the roofline engine;
bigger N amortizes its ~260ns per-instruction overhead, and the
software-pipeline skew across batch boundaries keeps ACT gapless).
Per batch the chunks run in two passes over the s/u quarters --
(q=0,1 for all stripes) then (q=2,3) -- so each output half's epilogue
runs as soon as its accumulation chains stop, hiding the tail.

PSUM budget (8 banks):
  g tiles [128, 3, 512] x2 bufs -- 6 banks, ping-pong
  pz      [128, 2, 512]         -- 2 banks, Z^T col-packed: 4 interleaved
          accumulation chains keyed by the s/u quarter (has_written clear
          is per-partition, verified on HW, so chains may share banks)
PE packing: mm1 (K=64) alternates row groups 0/64 per stripe.
The prologue (x load -> bf16 dup -> DRAM round-trip -> DMA-xbar X^T)
is pipelined at t-quarter granularity to cut batch-0 latency.

Sharding: pure data-parallel over batch B=32 -> 4 batches per core x 8.
"""

import os
import sys

import numpy as np

sys.path.insert(0, "/opt/trn_rl_repo")

import concourse.bass as bass
import concourse.tile as tile
from concourse import bacc, mybir
from concourse.bass_utils import run_bass_kernel_spmd

FP32 = mybir.dt.float32
BF16 = mybir.dt.bfloat16
FP16 = mybir.dt.float16

B, T, C = 32, 2048, 64
N_CORES = 8
BPC = B // N_CORES  # batches per core

# Stashed by kernel() for the test harness (exec time etc.)
LAST_RESULTS = None


def _body(ctx, tc, out_ap, x_ap, r, bpc, t, dbg=False):
    """Emit the per-core kernel IR.

    out_ap/x_ap: DRAM APs of shape [bpc, t, C].
    r: python float (r_sigma value, baked as immediates).
    """
    nc = tc.nc

    def dump(name, sb_ap, dt=None):
        if not dbg:
            return
        d = nc.dram_tensor(
            name, list(sb_ap.shape), dt or sb_ap.dtype, kind="ExternalOutput"
        ).ap()
        nc.sync.dma_start(out=d, in_=sb_ap)

    nt = t // 128          # t-blocks (stripes) per batch
    nq = nt // 4           # t-blocks per quarter
    exp2r = 2.0 * r

    # degree-7 polynomial fit of e^z on [-2, 2] (|z|=|2r G| stays < ~1 for
    # this problem's r=0.01, randn x); used by the DVE-offloaded exp tiles.
    # Horner via scalar_tensor_tensor: p=a7*z; p=(p+a_k)*z ...; out=p+a0.
    zg = np.linspace(-2.0, 2.0, 4001)
    pc_hi_first = np.polyfit(zg, np.exp(zg), 7)
    PA = [float(v) for v in pc_hi_first]  # [a7, a6, ..., a0]

    # chunk stream: per batch, pass 0 = quarters (0, 1), pass 1 = (2, 3)
    chunks = [
        (b, i, q)
        for b in range(bpc)
        for qp in range(2)
        for i in range(nt)
        for q in (2 * qp, 2 * qp + 1)
    ]
    nck = len(chunks)
    ntile = (nck + 2) // 3

    # tile index after whose mm2 each (batch, half) epilogue can run:
    # the tile containing that half's last chunk (i = nt-1)
    epi_after = {}
    for pos, (b, i, q) in enumerate(chunks):
        if i == nt - 1 and q % 2 == 1:
            epi_after[pos // 3] = (b, q // 2)
    # stream position at which to emit each batch's prologue (lookahead)
    prolog_at = {}
    for b in range(bpc):
        first = min(pos for pos, c in enumerate(chunks) if c[0] == b) // 3
        # lookahead 8 keeps each prologue's DVE burst clear of the epilogue
        # tiles in the DVE FIFO (12 was measured worse for exactly that)
        prolog_at[b] = max(0, first - 8)

    # exp tiles offloaded from ACT (roofline engine) to a DVE polynomial;
    # spaced out and kept away from epilogue tiles so the DVE FIFO never
    # delays an epilogue's PSUM drain.
    avoid = set()
    for e in epi_after:
        avoid.update((e - 1, e, e + 1, e + 2))
    # (measured: a full-tile DVE polynomial blob (~9us) exceeds the pipeline
    # skew and stalls mm2 -- net large loss, so the offload stays disabled)
    dve_tiles = set()

    # SBUF pools (2 slots per tag for cross-batch pipelining)
    xpool = ctx.enter_context(tc.tile_pool(name="x32", bufs=2))
    xxpool = ctx.enter_context(tc.tile_pool(name="xx", bufs=2))
    sqpool = ctx.enter_context(tc.tile_pool(name="sq", bufs=2))
    ypool = ctx.enter_context(tc.tile_pool(name="yb", bufs=2))
    xbpool = ctx.enter_context(tc.tile_pool(name="xbp", bufs=2))
    xtpool = ctx.enter_context(tc.tile_pool(name="xt", bufs=2))
    apool = ctx.enter_context(tc.tile_pool(name="a0", bufs=6))
    opool = ctx.enter_context(tc.tile_pool(name="osb", bufs=2))
    zpool = ctx.enter_context(tc.tile_pool(name="zply", bufs=2))
    # PSUM: 2 ping-pong G tiles of 3 banks + col-packed Z^T accum (2 banks)
    gpool = ctx.enter_context(tc.tile_pool(name="gps", bufs=2, space="PSUM"))
    ppool = ctx.enter_context(tc.tile_pool(name="pps", bufs=1, space="PSUM"))
    # DRAM scratch for the bf16 transpose round-trip
    dpool = ctx.enter_context(tc.tile_pool(name="dsc", bufs=2, space="DRAM"))

    pz = ppool.tile([128, 2, 512], FP32)    # 2 banks, Z^T col-packed

    # per-batch state, filled by prologue(b)
    st = [None] * bpc

    def prologue(b):
        xb_dram = x_ap[b].rearrange("(k p) c -> p k c", p=128)  # [128, nt, C]
        x32, xth = [], []
        for u in range(4):
            xh = xpool.tile([128, nq, C], FP32, tag=f"x32{u}")
            nc.sync.dma_start(out=xh[:], in_=xb_dram[:, u * nq : (u + 1) * nq])
            x32.append(xh)
            # bf16 x, written twice side by side, so one DMA-xbar transpose
            # yields X^T duplicated on both partition halves (mm1 row groups)
            xbp = xbpool.tile([128, nq, 2 * C], BF16, tag=f"xbp{u}")
            nc.vector.tensor_copy(xbp[:, :, 0:C], xh[:])
            nc.vector.tensor_copy(xbp[:, :, C : 2 * C], xh[:])
            xbd = dpool.tile([t // 4, 2 * C], BF16, tag=f"xbd{u}")
            nc.sync.dma_start(
                out=xbd.rearrange("(k p) c -> p k c", p=128), in_=xbp[:]
            )
            xt = xtpool.tile([128, t // 4], BF16, tag=f"xt{u}")
            nc.sync.dma_start_transpose(out=xt[:], in_=xbd[:])
            xth.append(xt)
            # xt[c, tt] = xt[64+c, tt] = x[u*512 + tt, c] for c < 64.

        # row stats per quarter -- sq/ev/yb chains stay quarter-local so the
        # first mm2 never waits on the whole batch's x to load.  They run on
        # the otherwise-idle GpSimd engine to keep the DVE queue short.
        ev, yb = [], []
        for u in range(4):
            xx = xxpool.tile([128, nq, C], FP32, tag=f"xx{u % 2}")
            nc.vector.tensor_mul(xx[:], x32[u][:], x32[u][:])
            sq = sqpool.tile([128, nq], FP32, tag=f"sq{u}")
            nc.vector.tensor_reduce(
                sq[:],
                xx[:],
                axis=mybir.AxisListType.X,
                op=mybir.AluOpType.add,
            )
            evu = sqpool.tile([128, nq], FP32, tag=f"ev{u}")
            nc.scalar.activation(
                evu[:], sq[:], mybir.ActivationFunctionType.Exp, scale=-r
            )
            ev.append(evu)
            ybu = ypool.tile([128, nq, C], BF16, tag=f"yb{u}")
            for k in range(nq):
                nc.vector.tensor_scalar_mul(
                    ybu[:, k], x32[u][:, k], evu[:, k : k + 1]
                )
            yb.append(ybu)
        st[b] = (x32, xth, ev, yb)
        if dbg and b == 0:
            dump("dbg_ev", ev[0][:])
            dump("dbg_yb", yb[0][:])
            dump("dbg_xt", xth[0][:])

    a0_tiles = [None] * ntile
    g_tiles = [None] * ntile

    def nch(k):
        return min(3, nck - 3 * k)

    def mm1(k):
        g = gpool.tile([128, 3, 512], FP32)
        g_tiles[k] = g
        for j in range(nch(k)):
            b, i, q = chunks[3 * k + j]
            xth = st[b][1]
            rows = 64 * (i % 2)
            xl = xth[i // nq]         # lhsT t-block quarter
            nc.tensor.matmul(
                g[:, j],
                lhsT=xl[rows : rows + 64, (i % nq) * 128 : (i % nq + 1) * 128],
                rhs=xth[q][rows : rows + 64, :],
                start=True,
                stop=True,
                skip_group_check=True,
            )

    def act(k):
        n = nch(k)
        g = g_tiles[k]
        g_tiles[k] = None
        a0 = apool.tile([128, 3, 512], BF16)
        if k in dve_tiles:
            # DVE polynomial exp: z = 2r*g (frees the PSUM tile fast),
            # then Horner in fp16 at the DVE 2x rate.
            z = zpool.tile([128, 3, 512], FP16, tag="z")
            nc.vector.tensor_scalar_mul(z[:, 0:n], g[:, 0:n], exp2r)
            pa = zpool.tile([128, 3, 512], FP16, tag="pa")
            pb = zpool.tile([128, 3, 512], FP16, tag="pb")
            nc.vector.tensor_scalar_mul(pa[:, 0:n], z[:, 0:n], PA[0])
            cur, nxt = pa, pb
            for a_k in PA[1:7]:
                nc.vector.scalar_tensor_tensor(
                    nxt[:, 0:n],
                    in0=cur[:, 0:n],
                    scalar=a_k,
                    in1=z[:, 0:n],
                    op0=mybir.AluOpType.add,
                    op1=mybir.AluOpType.mult,
                )
                cur, nxt = nxt, cur
            nc.vector.tensor_scalar_add(a0[:, 0:n], cur[:, 0:n], PA[7])
        else:
            nc.scalar.activation(
                a0[:, 0:n],
                g[:, 0:n],
                mybir.ActivationFunctionType.Exp,
                scale=exp2r,
            )
        a0_tiles[k] = a0
        if dbg and k == 0:
            gsb = xxpool.tile([128, 3, 512], FP32, tag="gdump")
            nc.vector.tensor_copy(gsb[:], g[:])
            dump("dbg_g00", gsb[:])
            dump("dbg_a00", a0[:])

    def mm2(k):
        a0 = a0_tiles[k]
        a0_tiles[k] = None
        for j in range(nch(k)):
            b, i, q = chunks[3 * k + j]
            yb = st[b][3]
            # chain q: parts 64*(q//2).., bank q%2; start/stop per stripe
            nc.tensor.matmul(
                pz[64 * (q // 2) : 64 * (q // 2) + 64, q % 2],
                lhsT=yb[i // nq][:, i % nq],
                rhs=a0[:, j],
                start=(i == 0),
                stop=(i == nt - 1),
                skip_group_check=True,
            )

    # ---- epilogue (one output half, two u-quarters pipelined):
    # Z^T -> bf16 -> DMA-xbar transpose -> e_u * Z + x -> DRAM.
    # tr[p, jj, c] = zq[64h+c, jj*128+p] = Z[u, c] for u-block 4*q+jj
    def epilogue(b, h):
        x32, _, ev, _ = st[b]
        ob_dram = out_ap[b].rearrange("(k p) c -> p k c", p=128)
        for n in range(2):
            q = 2 * h + n
            zq = opool.tile([128, 512], BF16, tag=f"zt{q}")
            nc.vector.tensor_copy(
                zq[64 * h : 64 * h + 64, :], pz[64 * h : 64 * h + 64, n]
            )
            tr = opool.tile([128, nq, C], BF16, tag=f"tr{q}")
            nc.sync.dma_start_transpose(
                out=tr[:], in_=zq[64 * h : 64 * h + 64, :]
            )
            osb = opool.tile([128, nq, C], FP32, tag=f"osb{q}")
            for jj in range(nq):
                j = q * nq + jj
                nc.vector.scalar_tensor_tensor(
                    osb[:, jj],
                    in0=tr[:, jj],
                    scalar=ev[j // nq][:, j % nq : j % nq + 1],
                    in1=x32[j // nq][:, j % nq],
                    op0=mybir.AluOpType.mult,
                    op1=mybir.AluOpType.add,
                )
            nc.sync.dma_start(
                out=ob_dram[:, q * nq : (q + 1) * nq], in_=osb[:]
            )

    # ---- main stream, software-pipelined emission: step k emits
    # mm2(k-2) -> mm1(k) -> ACT(k-1) so the PE queue head never blocks
    # on fresh work, including across batch boundaries. ----
    for k in range(ntile + 3):
        for b, pos in prolog_at.items():
            if pos == k:
                prologue(b)
        if k >= 3:
            mm2(k - 3)
            if k - 3 in epi_after:
                epilogue(*epi_after[k - 3])
        if k < ntile:
            mm1(k)
        if 1 <= k < ntile + 1:
            act(k - 1)


def build(r, bpc=BPC, t=T, dbg=False):
    """Build + compile the Bass module for one core's shard."""
    from contextlib import ExitStack

    nc = bacc.Bacc(
        "TRN2", target_bir_lowering=False, debug=False, num_devices=N_CORES
    )
    x_ap = nc.dram_tensor("x", [bpc, t, C], FP32, kind="ExternalInput").ap()
    out_ap = nc.dram_tensor("out", [bpc, t, C], FP32, kind="ExternalOutput").ap()
    with tile.TileContext(nc) as tc:
        with ExitStack() as ctx:
            _body(ctx, tc, out_ap, x_ap, r, bpc, t, dbg=dbg)
    nc.compile()
    return nc


def kernel(x, r_sigma):
    global LAST_RESULTS
    x = np.ascontiguousarray(np.asarray(x, dtype=np.float32))
    r = float(np.asarray(r_sigma).reshape(-1)[0])
    assert x.shape == (B, T, C), x.shape

    nc = build(r)
    in_maps = [
        {"x": np.ascontiguousarray(x[i * BPC : (i + 1) * BPC])}
        for i in range(N_CORES)
    ]
    trace = bool(int(os.environ.get("KERNEL_TRACE", "0")))
    res = run_bass_kernel_spmd(
        nc, in_maps, core_ids=list(range(N_CORES)), trace=trace
    )
    LAST_RESULTS = res
    out = np.concatenate([res.results[i]["out"] for i in range(N_CORES)], axis=0)
    return out.astype(np.float32)
